# revision 13
# baseline (speedup 1.0000x reference)
"""BiCutLoss Trainium2 kernel (8-core data parallel over batch).

Reference semantics (B=16384, L=1024):
    temp[b,j]  = argmax(output[b,j,:])          # 1 iff out1 > out0 (ties -> 0)
    idx[b]     = L if row all-ones else index of last zero
    mask[b,j]  = j <= idx[b]
    r1[b,j]    = -1/log2(j+2)  if labels==1 else (j+1)/alpha
    loss       = sum(output[...,1] * mask * r1) / B

Restructuring: masked_sum = full_sum - tail_sum, with the tail (j > idx)
confined to the last W=32 columns whenever each row has a zero decision
there (P(violation) = 2^-32 per row; a per-row flag catches it and the
host falls back to exact numpy, so the kernel stays correct for all
inputs).

v5 engine plan (measured costs, [128,2048] bf16 chunks):
  * TRANSPOSED main stream: out1.T [1024(j), 2048(b)] bf16, 8 chunks.
    With j on partitions, the reward weights Bv[j]/D[j] are
    per-partition scalars every engine can apply.
  * DVE is the binding engine: ql_c = out1_c*lab_c (mixed bf16*u8,
    2.28us) x8 plus the window mask chain. Window ops are interleaved
    between the early qls so cross-engine latency hides inside ql time.
  * term1 (sum Bv*out1): PE matmuls, bv_col stationary, all 32
    accumulated into ONE [1,512] PSUM; ScalarE Identity+accum drains.
  * term2 (sum D*ql): ScalarE activation scale=d_col + accum_out for
    chunks 0-6 (2.08us each); chunk 7 via PE d_col-matmuls.
  * GpSimd: label DMAs + the pure-bf16 window multiplies (gi, u, q2).
  * Window (W=32, packed [128, 16*32]): wdiff = out0w-out1w in f32
    (f32 subtract has exact sign -> decisions match the reference),
    out1w bf16, labw u8 cast-DMA'd to bf16. All small constants ride
    in one packed bf16 tensor; outputs merge into one [128,28] f32.
"""

import threading
from contextlib import ExitStack

import numpy as np

B, L = 16384, 1024
N_CORES = 8
ROWS_PER_CORE = B // N_CORES  # 2048
ALPHA = 0.65
W = 32  # tail window width
N_SEG = ROWS_PER_CORE // 128  # 16 window segments per partition
N_CHUNKS = L // 128  # 8 transposed chunks

_compiled = threading.local()


def _reward_rows():
    j = np.arange(L, dtype=np.float64)
    bv = (j + 1.0) / ALPHA
    d = -1.0 / np.log2(j + 2.0) - bv
    return bv, d


def _build(num_devices=N_CORES):
    import concourse.tile as tile
    from concourse import bacc, mybir

    f32 = mybir.dt.float32
    bf16 = mybir.dt.bfloat16
    u8 = mybir.dt.uint8
    Alu = mybir.AluOpType
    Act = mybir.ActivationFunctionType
    Ax = mybir.AxisListType

    RB = ROWS_PER_CORE  # 2048 batch rows per core
    WSEG = N_SEG * W  # 512 packed window width
    CP = WSEG + 2 * W + 2 * N_CHUNKS  # packed const width: 592

    nc = bacc.Bacc(
        "TRN2",
        target_bir_lowering=False,
        debug=False,
        enable_asserts=True,
        num_devices=num_devices,
    )

    out1T_d = nc.dram_tensor("out1T", [L, RB], bf16, kind="ExternalInput").ap()
    labT_d = nc.dram_tensor("labT", [L, RB], u8, kind="ExternalInput").ap()
    wdiff_d = nc.dram_tensor("wdiff", [128, WSEG], f32, kind="ExternalInput").ap()
    w1_d = nc.dram_tensor("w1", [128, WSEG], bf16, kind="ExternalInput").ap()
    labw_d = nc.dram_tensor("labw", [128, WSEG], u8, kind="ExternalInput").ap()
    cpack_d = nc.dram_tensor("cpack", [128, CP], bf16, kind="ExternalInput").ap()
    dcol32_d = nc.dram_tensor("dcol32", [128, N_CHUNKS], f32, kind="ExternalInput").ap()

    accs_d = nc.dram_tensor("accs", [128, 28], f32, kind="ExternalOutput").ap()

    with tile.TileContext(nc) as tc, ExitStack() as ctx:
        const = ctx.enter_context(tc.tile_pool(name="const", bufs=1))
        inp = ctx.enter_context(tc.tile_pool(name="inp", bufs=4))
        lpool = ctx.enter_context(tc.tile_pool(name="lpool", bufs=4))
        work = ctx.enter_context(tc.tile_pool(name="work", bufs=4))
        junkp = ctx.enter_context(tc.tile_pool(name="junkp", bufs=4))
        psum = ctx.enter_context(tc.tile_pool(name="psum", bufs=1, space="PSUM"))

        # ---- input DMA issues: main streams first, small stuff behind ----
        out1_c = []
        for f in range(4):
            t = inp.tile([128, 2, RB], bf16, tag="out1c")
            nc.sync.dma_start(
                t[:],
                out1T_d[256 * f : 256 * (f + 1), :].rearrange(
                    "(k p) b -> p k b", p=128
                ),
            )
            out1_c.append(t)
        lab_c = []
        for f in range(4):
            t = lpool.tile([128, 2, RB], u8, tag="labc")
            nc.gpsimd.dma_start(
                t[:],
                labT_d[256 * f : 256 * (f + 1), :].rearrange(
                    "(k p) b -> p k b", p=128
                ),
            )
            lab_c.append(t)
        wdiff_t = const.tile([128, WSEG], f32)
        nc.sync.dma_start(wdiff_t[:], wdiff_d[:])
        w1_t = const.tile([128, WSEG], bf16)
        nc.sync.dma_start(w1_t[:], w1_d[:])
        labw_t = const.tile([128, WSEG], bf16)
        nc.gpsimd.dma_start(labw_t[:], labw_d[:])  # cast u8 -> bf16 (tiny)
        cpack_t = const.tile([128, CP], bf16)
        nc.sync.dma_start(cpack_t[:], cpack_d[:])
        dcol32_t = const.tile([128, N_CHUNKS], f32)
        nc.sync.dma_start(dcol32_t[:], dcol32_d[:])

        gio = cpack_t[:, 0:WSEG]
        bvw_t = cpack_t[:, WSEG : WSEG + W]
        dw_t = cpack_t[:, WSEG + W : WSEG + 2 * W]
        bvcol_t = cpack_t[:, WSEG + 2 * W : WSEG + 2 * W + N_CHUNKS]
        dcol16_t = cpack_t[:, WSEG + 2 * W + N_CHUNKS : CP]

        accs_t = const.tile([128, 28], f32)
        nc.vector.memset(accs_t[:], 0.0)

        psA = psum.tile([1, 512], f32)  # term1
        psB = psum.tile([1, 512], f32)  # term2, chunk 7
        Q = RB // 512

        seg3 = lambda ap: ap.rearrange("p (s w) -> p s w", w=W)
        last0 = const.tile([128, N_SEG], bf16)
        l0p = const.tile([128, N_SEG], bf16)
        ge = work.tile([128, WSEG], bf16, tag="ge")
        gi = work.tile([128, WSEG], bf16, tag="gi")
        tm = work.tile([128, WSEG], bf16, tag="tm")
        u = work.tile([128, WSEG], bf16, tag="u")
        q2 = work.tile([128, WSEG], bf16, tag="q2")
        bvw_bc = (
            bvw_t.rearrange("p (s w) -> p s w", s=1).to_broadcast((128, N_SEG, W))
        )
        dw_bc = (
            dw_t.rearrange("p (s w) -> p s w", s=1).to_broadcast((128, N_SEG, W))
        )

        qls = []

        def emit_ql(c):
            ot = out1_c[c // 2][:, c % 2, :]
            lb = lab_c[c // 2][:, c % 2, :]
            ql = junkp.tile([128, RB], bf16, tag="ql")
            nc.vector.tensor_tensor(ql[:], ot, lb, Alu.mult)
            qls.append(ql)

        def emit_window_piece(step):
            if step == 0:
                nc.vector.tensor_scalar(ge[:], wdiff_t[:], 0.0, None, Alu.is_ge)
                nc.gpsimd.tensor_tensor(gi[:], ge[:], gio, Alu.mult)
            elif step == 1:
                nc.vector.tensor_reduce(last0[:], seg3(gi[:]), Ax.X, Alu.max)
                nc.vector.tensor_scalar(
                    accs_t[:, 12 : 12 + N_SEG], last0[:], 0.5, None, Alu.is_le
                )
                nc.vector.scalar_tensor_tensor(
                    l0p[:], accs_t[:, 12 : 12 + N_SEG], float(W), last0[:],
                    Alu.mult, Alu.add,
                )
                nc.vector.tensor_tensor(
                    seg3(tm[:]), seg3(gio),
                    l0p[:].to_broadcast((128, N_SEG, W)), Alu.is_gt,
                )
                nc.gpsimd.tensor_tensor(u[:], tm[:], w1_t[:], Alu.mult)
                nc.gpsimd.tensor_tensor(q2[:], u[:], labw_t[:], Alu.mult)
            elif step == 2:
                junk1 = junkp.tile([128, WSEG], bf16, tag="junkw")
                nc.vector.scalar_tensor_tensor(
                    seg3(junk1[:]), seg3(u[:]), 1.0, bvw_bc, Alu.mult, Alu.mult,
                    accum_out=accs_t[:, 8:9],
                )
                junk2 = junkp.tile([128, WSEG], bf16, tag="junkw")
                nc.vector.scalar_tensor_tensor(
                    seg3(junk2[:]), seg3(q2[:]), 1.0, dw_bc, Alu.mult, Alu.mult,
                    accum_out=accs_t[:, 9:10],
                )

        # DVE order: ge, ql0, [gi], ql1, [mask chain], ql2, ql3, [tail stts],
        # ql4..ql7 — window pieces slot into ql gaps so their cross-engine
        # waits hide inside ql execution time.
        emit_window_piece(0)
        emit_ql(0)
        emit_ql(1)
        emit_window_piece(1)
        emit_ql(2)
        emit_ql(3)
        emit_window_piece(2)
        for c in range(4, N_CHUNKS):
            emit_ql(c)

        # PE: term1 matmuls for every chunk; term2 matmuls for chunk 7.
        a_seen = 0
        for c in range(N_CHUNKS):
            ot = out1_c[c // 2][:, c % 2, :]
            for k in range(Q):
                nc.tensor.matmul(
                    psA[:], bvcol_t[:, c : c + 1], ot[:, 512 * k : 512 * (k + 1)],
                    start=(a_seen == 0), stop=(a_seen == N_CHUNKS * Q - 1),
                )
                a_seen += 1
            if c == N_CHUNKS - 1:
                for k in range(Q):
                    nc.tensor.matmul(
                        psB[:], dcol16_t[:, c : c + 1],
                        qls[c][:, 512 * k : 512 * (k + 1)],
                        start=(k == 0), stop=(k == Q - 1),
                    )

        # ScalarE: term2 weighted row-sums for chunks 0-6, then PSUM drains.
        for c in range(N_CHUNKS - 1):
            junk = junkp.tile([128, RB], bf16, tag="junkact")
            nc.scalar.activation(
                junk[:], qls[c][:], Act.Identity,
                scale=dcol32_t[:, c : c + 1],
                accum_out=accs_t[:, c : c + 1],
            )
        ej_a = junkp.tile([1, 512], f32, tag="ejA")
        nc.scalar.activation(
            ej_a[:], psA[:], Act.Identity, accum_out=accs_t[0:1, 10:11]
        )
        ej_b = junkp.tile([1, 512], f32, tag="ejB")
        nc.scalar.activation(
            ej_b[:], psB[:], Act.Identity, accum_out=accs_t[0:1, 11:12]
        )
        nc.sync.dma_start(accs_d[:], accs_t[:])

    nc.compile()
    return nc


def _get_nc():
    if getattr(_compiled, "nc", None) is None:
        _compiled.nc = _build()
    return _compiled.nc


def _in_maps(output, labels):
    import ml_dtypes

    bf16 = ml_dtypes.bfloat16
    bv, dd = _reward_rows()
    rp = ROWS_PER_CORE
    WSEG = N_SEG * W
    lab8 = labels.astype(np.uint8)
    jcol = np.arange(L).reshape(N_CHUNKS, 128).T  # [128, 8]: j = 128*c + p
    # packed consts: [gio 512 | bvw 32 | dw 32 | bvcol 8 | dcol16 8]
    cpack = np.zeros((128, WSEG + 2 * W + 2 * N_CHUNKS), dtype=bf16)
    gio = np.tile(np.arange(1, W + 1, dtype=np.float64), N_SEG)
    cpack[:, 0:WSEG] = gio.astype(bf16)[None, :]
    cpack[:, WSEG : WSEG + W] = bv[L - W :].astype(bf16)[None, :]
    cpack[:, WSEG + W : WSEG + 2 * W] = dd[L - W :].astype(bf16)[None, :]
    cpack[:, WSEG + 2 * W : WSEG + 2 * W + N_CHUNKS] = bv[jcol].astype(bf16)
    cpack[:, WSEG + 2 * W + N_CHUNKS :] = dd[jcol].astype(bf16)
    dcol32 = dd[jcol].astype(np.float32)
    maps = []
    for c in range(N_CORES):
        o = output[c * rp : (c + 1) * rp]
        out1T = np.ascontiguousarray(o[:, :, 1].T, dtype=bf16)
        labT = np.ascontiguousarray(lab8[c * rp : (c + 1) * rp].T)
        w0 = o[:, L - W :, 0].astype(np.float32)
        w1f = o[:, L - W :, 1].astype(np.float32)
        wdiff = (w0 - w1f).reshape(128, -1)
        maps.append(
            {
                "out1T": out1T,
                "labT": labT,
                "wdiff": np.ascontiguousarray(wdiff),
                "w1": np.ascontiguousarray(w1f.reshape(128, -1).astype(bf16)),
                "labw": np.ascontiguousarray(
                    lab8[c * rp : (c + 1) * rp, L - W :].reshape(128, -1)
                ),
                "cpack": cpack,
                "dcol32": dcol32,
            }
        )
    return maps


def _host_fallback(output, labels):
    temp = output[:, :, 1] > output[:, :, 0]
    allones = temp.all(axis=1)
    z = ~temp
    last_zero = (L - 1) - np.argmax(z[:, ::-1], axis=1)
    idx = np.where(allones, L, last_zero)
    mask = np.arange(L)[None, :] <= idx[:, None]
    j = np.arange(L, dtype=np.float64)
    r1 = np.where(labels == 1, -1.0 / np.log2(j + 2.0), (j + 1.0) / ALPHA)
    return np.float32(
        (output[:, :, 1].astype(np.float64) * mask * r1).sum() / B
    )


def _combine(results, output, labels):
    total = 0.0
    suspicious = 0
    for c, r in enumerate(results):
        accs = np.asarray(r["accs"], dtype=np.float64)
        main = accs[0, 10] + accs[0, 11] + accs[:, 0:7].sum()
        tail = accs[:, 8].sum() + accs[:, 9].sum()
        total += main - tail
        # rows flagged "no zero decision in window": genuine all-ones rows
        # are fine (tail = 0) but a row whose last zero is before the
        # window is not — recheck on host. P = 2^-32 per row for random
        # inputs.
        flags = accs[:, 12 : 12 + N_SEG]
        if flags.max() > 0:
            rp = ROWS_PER_CORE
            o = output[c * rp : (c + 1) * rp]
            allones_rows = (o[:, :, 1] > o[:, :, 0]).all(axis=1)
            flagged = flags.reshape(-1) > 0  # row = p*16 + s
            suspicious += int((flagged & ~allones_rows).sum())
    if suspicious > 0:
        return _host_fallback(output, labels)
    return np.float32(total / B)


def kernel(output: np.ndarray, labels: np.ndarray) -> np.ndarray:
    from concourse.bass_utils import run_bass_kernel_spmd

    assert output.shape == (B, L, 2), output.shape
    nc = _get_nc()
    res = run_bass_kernel_spmd(
        nc, _in_maps(output, labels), core_ids=list(range(N_CORES))
    )
    return _combine(res.results, output, labels)
